# revision 2
# baseline (speedup 1.0000x reference)
"""BrightnessLoss Trainium2 kernel (raw Bass, 8-core data parallel).

reference:
    V(x)   = max_c(clip(x, 0, 1))        over channel dim (RGB)
    result = mean(|V(pred) - V(target)|) over (N, H, W)

Identities used on device:
    clip(max(r,g,b),0,1) == max_c(clip(x,0,1))          (clip is monotone)
    W := relu(1 - relu(m)) == 1 - clip(m, 0, 1)
    |Vp - Vt| == |Wp - Wt|
    sum|Wp - Wt| == 2*sum max(Wp,Wt) - sum(Wp + Wt)

Work is cut into "units" (image chunks along the plane's free dim). Both
sides (pred, targ) of a unit live in ONE sbuf slot tensor [P, 2, 3, w] so
each DVE/ACT op covers both sides in a single wide instruction:
    DVE  TT   m = max(R2, G2)            [P, 2, w]
    DVE  STT  u = max(max(m,0), B2)      [P, 2, w]
    ACT       W = Relu(1 - u)  (bf16), accum_out = sum(Wp)+sum(Wt)
    DVE  STT  max(Wp, Wt) bf16 (2x mode), accum_out = sum
Input DMAs ride two HWDGE rings (Sync=even units, ACT=odd units) with
S_IN=4 slots so each ring stays ~2 transfers deep and never starves on
compute. The last image's final chunk shrinks (640/256/128) so the
post-DMA dependency chain of the very last unit is short. Partials go out
in two DMAs (bulk early, last units at the end). Host combines in float64.
"""

import numpy as np

N_CORES = 8
N_IMG = 4  # 32 / 8
C = 3
P = 128
F = 2048  # 512*512 / 128
N_PIX = 32 * 512 * 512
N_CHUNKS = 2  # chunks per plane
S_IN = 4  # input slot depth (units in flight)
TAIL_SPLIT = (640, 256, 128)  # last image final-chunk split (sums to F/N_CHUNKS)


def _plan_units(n_img, f, n_chunks, tail_split):
    """Units: (img, col_offset, width). Last image's final chunk is split
    further per tail_split to shorten the end-of-kernel dependency chain."""
    fc = f // n_chunks
    units = []
    for img in range(n_img):
        offs = [(j * fc, fc) for j in range(n_chunks)]
        if img == n_img - 1 and tail_split:
            assert sum(tail_split) == fc
            off0 = offs[-1][0]
            offs = offs[:-1]
            o = off0
            for w in tail_split:
                offs.append((o, w))
                o += w
        for off, w in offs:
            units.append((img, off, w))
    return units, fc


def _build_program(n_img=N_IMG, f=F, n_chunks=N_CHUNKS, tail_split=TAIL_SPLIT):
    from contextlib import ExitStack

    import concourse.bass as bass
    import concourse.mybir as mybir

    fp32 = mybir.dt.float32
    bf16 = mybir.dt.bfloat16
    Alu = mybir.AluOpType
    Act = mybir.ActivationFunctionType

    assert f % n_chunks == 0
    units, fc = _plan_units(n_img, f, n_chunks, tail_split)
    n_units = len(units)

    # detect_race_conditions=False: the raw-mode CoreSim race detector can't
    # see same-engine program-order (DVE m1 -> STT RAW); hardware engines
    # execute in order.
    # The construction-time all_engine_barrier orders the const-tile memsets
    # against engines that read them; this kernel uses only instruction
    # immediates, so skip it and let the engines reach first work sooner.
    _orig_barrier = bass.Bass.all_engine_barrier
    bass.Bass.all_engine_barrier = lambda *a, **k: None
    try:
        nc = bass.Bass(
            "TRN2",
            target_bir_lowering=False,
            debug=False,
            detect_race_conditions=False,
        )
    finally:
        bass.Bass.all_engine_barrier = _orig_barrier
    pred = nc.dram_tensor("pred", [n_img, C, P, f], fp32, kind="ExternalInput").ap()
    targ = nc.dram_tensor("target", [n_img, C, P, f], fp32, kind="ExternalInput").ap()
    out = nc.dram_tensor(
        "partials", [P, 2 * n_units], fp32, kind="ExternalOutput"
    ).ap()

    with ExitStack() as ctx:
        sb = lambda name, shape, dt=fp32: ctx.enter_context(
            nc.sbuf_tensor(name, shape, dt)
        )
        sem = lambda name: ctx.enter_context(nc.semaphore(name))

        # one slot holds BOTH sides: [P, side, chan, fc]
        inb = [sb(f"in{s}", [P, 2, C, fc]) for s in range(S_IN)]
        ub = [sb(f"u{s}", [P, 2 * fc]) for s in range(2)]
        wb = [sb(f"w{s}", [P, 2 * fc], bf16) for s in range(2)]
        m1 = sb("m1", [P, 2 * fc])
        scr = sb("stt_scratch", [P, fc], bf16)
        acc = sb("acc", [P, 2 * n_units])

        inp_sem = [sem("inp0"), sem("inp1")]  # pred side, by ring parity
        int_sem = [sem("int0"), sem("int1")]  # targ side, by ring parity
        u_sem = sem("u")  # +1 per unit after DVE STT (inb consumed)
        act_sem = sem("act")  # +1 per unit after ACT (ub consumed, wb+acc ready)
        gp_sem = sem("gp")  # +1 per unit after DVE accum (wb consumed)
        out_sem = sem("outd")

        def dma_in(eng, side_idx, u):
            img, off, w = units[u]
            side = (pred, targ)[side_idx]
            s_sem = (inp_sem, int_sem)[side_idx]
            src = side[img, :, :, off : off + w].rearrange("c p f -> p c f")
            eng.dma_start(
                out=inb[u % S_IN][:, side_idx, :, :w],
                in_=src,
            ).then_inc(s_sem[u % 2], 16)

        block = ctx.enter_context(nc.Block(no_gpsimd_drain=True))

        @block.sync
        def _(sync):
            # even units ride the SP ring; odd units are issued from the ACT
            # stream (second HWDGE ring)
            for u in range(0, n_units, 2):
                if u >= S_IN:
                    # WAR inb[u%S_IN]: unit u-S_IN's STT was its last reader
                    sync.wait_ge(u_sem, u - S_IN + 1)
                dma_in(sync, 0, u)
                dma_in(sync, 1, u)
            if n_units > 2:
                # bulk of partials early; only the last 2 units' cols remain.
                # gp_sem >= k implies act_sem >= k (accum u waits ACT u), so
                # both engines' acc columns for units < k are final.
                sync.wait_ge(gp_sem, n_units - 2)
                sync.dma_start(
                    out=out[:, : 2 * (n_units - 2)],
                    in_=acc[:, : 2 * (n_units - 2)],
                ).then_inc(out_sem, 16)
            sync.wait_ge(gp_sem, n_units)
            # No out_sem wait after the final write: the block-exit drain
            # fences the HWDGE ring before NEFF completion.
            sync.dma_start(
                out=out[:, 2 * max(0, n_units - 2) :],
                in_=acc[:, 2 * max(0, n_units - 2) :],
            ).then_inc(out_sem, 16)

        @block.vector
        def _(vector):
            def accum(u):
                # max(Wp, Wt) elementwise (bf16 2x), accum_out = row sum
                w = units[u][2]
                vector.wait_ge(act_sem, u + 1)
                vector.scalar_tensor_tensor(
                    scr[:, :w],
                    wb[u % 2][:, :w],
                    0.0,
                    wb[u % 2][:, w : 2 * w],
                    op0=Alu.bypass,
                    op1=Alu.max,
                    accum_out=acc[:, 2 * u : 2 * u + 1],
                ).then_inc(gp_sem, 1)

            for u in range(n_units):
                w = units[u][2]
                t = inb[u % S_IN]
                vector.wait_ge(inp_sem[u % 2], 16 * (u // 2 + 1))
                vector.wait_ge(int_sem[u % 2], 16 * (u // 2 + 1))
                mv = m1[:, : 2 * w].rearrange("p (s w) -> p s w", s=2)
                uv = ub[u % 2][:, : 2 * w].rearrange("p (s w) -> p s w", s=2)
                vector.tensor_max(mv, t[:, :, 0, :w], t[:, :, 1, :w])
                if u >= 2:
                    # WAR on ub[u%2]: ACT's W of unit u-2 (its reader)
                    vector.wait_ge(act_sem, u - 1)
                vector.scalar_tensor_tensor(
                    uv,
                    mv,
                    0.0,
                    t[:, :, 2, :w],
                    op0=Alu.max,
                    op1=Alu.max,
                ).then_inc(u_sem, 1)
                if u > 0:
                    accum(u - 1)
            accum(n_units - 1)

        @block.scalar
        def _(scalar):
            # odd units' input DMAs ride the ACT HWDGE ring. Units 1 and 3 go
            # up front (fresh slots, no WAR); unit n+4 is placed right after
            # ACT(n), whose u_sem wait (>= n+1) covers the WAR for slot
            # (n+4) % S_IN (last STT reader was unit n).
            for u in (1, 3):
                if u < n_units:
                    dma_in(scalar, 0, u)
                    dma_in(scalar, 1, u)
            for n in range(n_units):
                w = units[n][2]
                scalar.wait_ge(u_sem, n + 1)
                if n >= 2:
                    # WAR on wb[n%2]: accum of unit n-2 (its reader)
                    scalar.wait_ge(gp_sem, n - 1)
                scalar.activation(
                    wb[n % 2][:, : 2 * w],
                    ub[n % 2][:, : 2 * w],
                    Act.Relu,
                    bias=1.0,
                    scale=-1.0,
                    accum_out=acc[:, 2 * n + 1 : 2 * n + 2],
                ).then_inc(act_sem, 1)
                if n + 4 < n_units and (n + 4) % 2 == 1:
                    dma_in(scalar, 0, n + 4)
                    dma_in(scalar, 1, n + 4)

        # Skip the Block-exit all-engine barrier (~4.3us): every cross-engine
        # dependency is semaphore-gated and the per-engine exit drains
        # (no_gpsimd_drain path) still fence the DMA rings, so engines may
        # halt independently — NEFF completion waits for all engines anyway.
        nc.all_engine_barrier = lambda *a, **k: None

    del nc.all_engine_barrier  # restore class method
    return nc


_program = None


def _get_program():
    global _program
    if _program is None:
        _program = _build_program()
    return _program


def _finish(partials_list):
    """partials_list: per-core [P, 2*n_units] f32 with cols per unit:
    [sum max(Wp,Wt), sum Wp + sum Wt].
    sum|Vp-Vt| = 2*sum(max) - (sum Wp + sum Wt)."""
    total = np.float64(0.0)
    for p in partials_list:
        p = p.astype(np.float64)
        total += 2.0 * p[:, 0::2].sum() - p[:, 1::2].sum()
    return np.array(total / N_PIX, dtype=np.float32)


def kernel(pred: np.ndarray, target: np.ndarray) -> np.ndarray:
    from concourse.bass_utils import run_bass_kernel_spmd

    nc = _get_program()
    pred = np.ascontiguousarray(pred, dtype=np.float32).reshape(
        N_CORES, N_IMG, C, P, F
    )
    target = np.ascontiguousarray(target, dtype=np.float32).reshape(
        N_CORES, N_IMG, C, P, F
    )
    in_maps = [{"pred": pred[i], "target": target[i]} for i in range(N_CORES)]
    res = run_bass_kernel_spmd(nc, in_maps, list(range(N_CORES)))
    return _finish([r["partials"] for r in res.results])
